# revision 13
# baseline (speedup 1.0000x reference)
"""DAU (dense_cnn) Trainium2 kernel: 8-way batch-parallel, one sample per core.

Layout: split-spatial-halves. Feature maps live as [128, ...] SBUF tiles where
partitions 0-63 hold channels 0-63 for image rows 0-63 ("half0") and partitions
64-127 hold channels 0-63 for rows 64-127 ("half1"). 3x3 convs run as 9
accumulating bf16 matmuls per 512-px chunk, with the two halves issued to
disjoint PE quadrants (tile_position (0,0) / (64,64)) so they execute
concurrently. Conv inputs sit in zero/edge-padded buffers [128, 66, 130]
(1-px halo; cross-half halo rows exchanged by small SBUF-SBUF DMAs).
"""
import numpy as np
import ml_dtypes
from contextlib import ExitStack

import concourse.bass as bass
import concourse.tile as tile
from concourse import bacc, mybir
from concourse.bass_utils import run_bass_kernel_spmd

F32 = mybir.dt.float32
BF16 = mybir.dt.bfloat16
bf = ml_dtypes.bfloat16
AF = mybir.ActivationFunctionType
OP = mybir.AluOpType
AX = mybir.AxisListType

C = 64
H = W = 128
NP = H * W          # 16384 pixels
HNP = NP // 2       # 8192 per half
HP, WP = 66, 130    # padded buffer geometry (slot 0 / 65 = pad+halo rows)
NCHUNK = 16         # 512-px conv chunks per half
DCH = 8             # dyn-conv chunks (1024 px per half each)

# dyn-conv tap assignment: taps 0-5 on DVE, 6-8 on GPSIMD
DVE_TAPS = [0, 1, 2, 3, 4, 5]
GP_TAPS = [6, 7, 8]


def _conv_taps(nc, ps, wts, src, k, n_taps=9, wcol=64):
    """Emit 9-tap x 2-half accumulating matmuls for output chunk k (4 rows)."""
    for t in range(n_taps):
        dhi, dwi = t // 3, t % 3
        for half in range(2):
            b = half * 64
            rhs = src[b:b + 64, 4 * k + dhi:4 * k + dhi + 4, dwi:dwi + W]
            nc.tensor.matmul(
                ps[b:b + 64, :] if wcol == 64 else ps[:, :],
                wts[b:b + 64, t, :],
                rhs,
                start=(t == 0), stop=(t == n_taps - 1),
                tile_position=(b, b if wcol == 64 else 0),
                skip_group_check=True,
            )


def build_kernel(ctx: ExitStack, tc: tile.TileContext, io, rb_f: float, eb_f: float):
    nc = tc.nc
    pool = ctx.enter_context(tc.tile_pool(name="main", bufs=1))
    pp = ctx.enter_context(tc.tile_pool(name="ps", bufs=1, space="PSUM"))

    # ---- weight / small tiles (DMA'd once) ----
    wt = {}
    for name in ("wt1T", "wt2T", "wt3T", "s1T", "s2T", "g1T", "g2T"):
        wt[name] = pool.tile([128, 9, 64], BF16, name=name)
    rwT = pool.tile([128, 64], BF16)
    fwAT = pool.tile([128, 128], BF16)
    fwBT = pool.tile([128, 128], BF16)
    bands = pool.tile([128, 50, 128], BF16)
    a1T = pool.tile([64, 128], BF16)
    a2T = pool.tile([128, 64], BF16)
    ident = pool.tile([128, 64], BF16)
    b1p = pool.tile([128, 1], F32)
    b2p = pool.tile([64, 1], F32)
    g1bp = pool.tile([128, 1], F32)
    g2bp = pool.tile([128, 9], F32)
    for name, t in list(wt.items()) + [("rwT", rwT), ("fwAT", fwAT), ("fwBT", fwBT),
                                       ("bands", bands), ("a1T", a1T), ("a2T", a2T),
                                       ("ident", ident), ("b1p", b1p), ("b2p", b2p),
                                       ("g1bp", g1bp), ("g2bp", g2bp)]:
        nc.sync.dma_start(t[:], io[name][:])

    # ---- big buffers (tag overlays keep us under the SBUF budget) ----
    # tag "A": fsum (trans input) -> s1 -> f_e
    fsum = pool.tile([128, HP, WP], BF16, tag="A", name="fsum")
    # tag "big2" bufs=2: t1, ftr, fused2, then 8 rotating kern chunk buffers
    t1 = pool.tile([128, HP, WP], BF16, tag="big2", bufs=2, name="t1")
    ftr = pool.tile([128, HP, WP], BF16, tag="big2", bufs=2, name="ftr")
    fev = pool.tile([128, HP, WP], BF16, name="fev")
    fb2 = pool.tile([128, HP, WP], BF16, name="fb2")
    f_b3 = pool.tile([128, HNP], BF16, name="f_b3")
    g_bf = pool.tile([128, HNP], BF16, name="g_bf")

    # psum tags: conv(2) + g2(2) + acc(2 banks) + tp(2) = 8 banks
    def ps_conv():
        return pp.tile([128, 512], F32, tag="conv", bufs=2, name="ps_conv")

    def ps_g2():
        return pp.tile([128, 512], F32, tag="g2", bufs=2, name="ps_g2")

    # ---- phase A: border memsets for zero-padded conv inputs ----
    for buf in (fsum, t1, ftr, fev):
        nc.vector.memset(buf[0:64, 0, :], 0)
        nc.vector.memset(buf[64:128, HP - 1, :], 0)
        nc.vector.memset(buf[:, :, 0:1], 0)
        nc.vector.memset(buf[:, :, WP - 1:WP], 0)

    # ---- phase B: load inputs (cast to bf16), E1 fsum = f_event + f_blur ----
    tmpp = ctx.enter_context(tc.tile_pool(name="tmp", bufs=2))
    for c in range(4):
        tmp_e = tmpp.tile([128, 16, 128], F32, tag="tmp_e")
        tmp_f = tmpp.tile([128, 16, 128], F32, tag="tmp_f")
        for half in range(2):
            b = half * 64
            off = half * HNP + 2048 * c
            nc.sync.dma_start(tmp_e[b:b + 64], io["fe"][:, off:off + 2048]
                              .rearrange("p (r w) -> p r w", r=16))
            nc.sync.dma_start(tmp_f[b:b + 64], io["fb"][:, off:off + 2048]
                              .rearrange("p (r w) -> p r w", r=16))
        nc.scalar.activation(fev[:, 1 + 16 * c:17 + 16 * c, 1:129], tmp_e[:],
                             AF.Copy)
        nc.gpsimd.tensor_add(fsum[:, 1 + 16 * c:17 + 16 * c, 1:129],
                             tmp_e[:], tmp_f[:])
    # halo rows (cross-half) for fev + fsum
    for buf in (fev, fsum):
        nc.sync.dma_start(buf[0:64, HP - 1, :], buf[64:128, 1, :])
        nc.sync.dma_start(buf[64:128, 0, :], buf[0:64, HP - 2, :])

    # ---- Wt1: t1 = relu(conv(fsum)) ----
    for k in range(NCHUNK):
        ps = ps_conv()
        _conv_taps(nc, ps, wt["wt1T"], fsum, k)
        nc.scalar.activation(t1[:, 1 + 4 * k:5 + 4 * k, 1:129], ps[:], AF.Relu)
    nc.sync.dma_start(t1[0:64, HP - 1, :], t1[64:128, 1, :])
    nc.sync.dma_start(t1[64:128, 0, :], t1[0:64, HP - 2, :])

    # ---- Wt2: ftr = conv(t1), with per-chunk pixel sums for the SE gate ----
    accums = pool.tile([128, NCHUNK], F32)
    for k in range(NCHUNK):
        ps = ps_conv()
        _conv_taps(nc, ps, wt["wt2T"], t1, k)
        nc.scalar.activation(ftr[:, 1 + 4 * k:5 + 4 * k, 1:129], ps[:],
                             AF.Identity, accum_out=accums[:, k:k + 1])
    nc.sync.dma_start(ftr[0:64, HP - 1, :], ftr[64:128, 1, :])
    nc.sync.dma_start(ftr[64:128, 0, :], ftr[0:64, HP - 2, :])

    # ---- Wt3 + E6: fused2 = ftr + relu(conv(ftr)) ----
    fused2 = pool.tile([128, HNP], BF16, tag="big2", bufs=2, name="fused2")
    for k in range(NCHUNK):
        ps = ps_conv()
        _conv_taps(nc, ps, wt["wt3T"], ftr, k)
        nc.vector.scalar_tensor_tensor(
            fused2[:, 512 * k:512 * (k + 1)], ps[:], 0.0,
            ftr[:, 1 + 4 * k:5 + 4 * k, 1:129], op0=OP.max, op1=OP.add)

    # ---- SE MLP: w0 = sigmoid(A2 @ relu(A1 @ mean(ftr))) ----
    asum = pool.tile([128, 1], F32)
    nc.vector.tensor_reduce(asum[:], accums[:], axis=AX.X, op=OP.add)
    acc_hi = pool.tile([64, 1], F32)
    nc.sync.dma_start(acc_hi[:], asum[64:128, :])
    s64f = pool.tile([64, 1], F32)
    nc.vector.tensor_add(s64f[:], asum[0:64, :], acc_hi[:])
    s64b = pool.tile([64, 1], BF16)
    nc.vector.tensor_copy(s64b[:], s64f[:])          # 1/16384 folded into a1T
    ps_m1 = pp.tile([128, 1], F32, tag="conv", bufs=2, name="ps_m1")
    nc.tensor.matmul(ps_m1[:], a1T[:], s64b[:], start=True, stop=True)
    hvec = pool.tile([128, 1], BF16)
    nc.scalar.activation(hvec[:], ps_m1[:], AF.Relu, bias=b1p[:])
    ps_m2 = pp.tile([64, 1], F32, tag="conv", bufs=2, name="ps_m2")
    nc.tensor.matmul(ps_m2[:], a2T[:], hvec[:], start=True, stop=True)
    w0g = pool.tile([64, 1], F32)
    nc.scalar.activation(w0g[:], ps_m2[:], AF.Sigmoid, bias=b2p[:])
    w0p1 = pool.tile([128, 1], F32)
    nc.sync.dma_start(w0p1[0:64, :], w0g[:])
    nc.sync.dma_start(w0p1[64:128, :], w0g[:])
    nc.vector.tensor_scalar_add(w0p1[:], w0p1[:], 1.0)

    # ---- E5: fb2 = f_blur * (1 + w0)  (reload f_blur, scale-cast into fb2) ----
    for c in range(4):
        tmp_f = tmpp.tile([128, 16, 128], F32, tag="tmp_f")
        for half in range(2):
            b = half * 64
            off = half * HNP + 2048 * c
            nc.sync.dma_start(tmp_f[b:b + 64], io["fb"][:, off:off + 2048]
                              .rearrange("p (r w) -> p r w", r=16))
        nc.scalar.activation(fb2[:, 1 + 16 * c:17 + 16 * c, 1:129], tmp_f[:],
                             AF.Copy, scale=w0p1[:])

    # ---- Rw + E7: fb2 *= (1 + rb + rgb_att)  in-place ----
    for k in range(NCHUNK):
        ps = ps_conv()
        for half in range(2):
            b = half * 64
            nc.tensor.matmul(ps[b:b + 64, :], rwT[b:b + 64, :],
                             fused2[b:b + 64, 512 * k:512 * (k + 1)],
                             start=True, stop=True, tile_position=(b, b),
                             skip_group_check=True)
        nc.vector.scalar_tensor_tensor(
            fb2[:, 1 + 4 * k:5 + 4 * k, 1:129], ps[:], 1.0 + rb_f,
            fb2[:, 1 + 4 * k:5 + 4 * k, 1:129], op0=OP.add, op1=OP.mult)

    # ---- fb2 edge padding (mode="edge") ----
    nc.vector.tensor_copy(fb2[0:64, 0, 1:129], fb2[0:64, 1, 1:129])
    nc.vector.tensor_copy(fb2[64:128, HP - 1, 1:129], fb2[64:128, HP - 2, 1:129])
    nc.sync.dma_start(fb2[0:64, HP - 1, 1:129], fb2[64:128, 1, 1:129])
    nc.sync.dma_start(fb2[64:128, 0, 1:129], fb2[0:64, HP - 2, 1:129])
    nc.vector.tensor_copy(fb2[:, :, 0:1], fb2[:, :, 1:2])
    nc.vector.tensor_copy(fb2[:, :, WP - 1:WP], fb2[:, :, WP - 2:WP - 1])

    # ---- G1: g = relu(conv(fev) + g1b) ----
    for k in range(NCHUNK):
        ps = ps_conv()
        _conv_taps(nc, ps, wt["g1T"], fev, k)
        nc.scalar.activation(g_bf[:, 512 * k:512 * (k + 1)], ps[:], AF.Relu,
                             bias=g1bp[:])

    # ---- G2 + dynamic per-pixel conv, chunked (1024 px per half / chunk) ----
    accp = ctx.enter_context(tc.tile_pool(name="dyn", bufs=1))

    def g2_chunk(c):
        kern = pool.tile([128, 9, 1024], BF16, tag="big2", bufs=2,
                         name=f"kern{c}")
        for t in range(9):
            for s in range(2):
                ps = ps_g2()
                off = 1024 * c + 512 * s
                for half in range(2):
                    b = half * 64
                    nc.tensor.matmul(ps[b:b + 64, :], wt["g2T"][b:b + 64, t, :],
                                     g_bf[b:b + 64, off:off + 512],
                                     start=True, stop=True, tile_position=(b, b),
                                     skip_group_check=True)
                nc.scalar.activation(kern[:, t, 512 * s:512 * (s + 1)], ps[:],
                                     AF.Identity, bias=g2bp[:, t:t + 1])
        return kern

    def dyn_chunk(c, kern):
        acc = pp.tile([128, 1024], F32, tag="acc", bufs=1, name="acc")
        gp_acc = accp.tile([128, 1024], F32, tag="gp_acc")
        first_dve, first_gp = True, True
        for t in DVE_TAPS + GP_TAPS:
            dhi, dwi = t // 3, t % 3
            shift = fb2[:, 8 * c + dhi:8 * c + dhi + 8, dwi:dwi + W]
            kt = kern[:, t, :]
            if t in DVE_TAPS:
                if first_dve:
                    nc.vector.tensor_mul(acc[:], kt, shift)
                    first_dve = False
                else:
                    dtmp = accp.tile([128, 1024], BF16, tag="dtmp", bufs=2)
                    nc.vector.tensor_mul(dtmp[:], kt, shift)
                    nc.vector.tensor_add(acc[:], acc[:], dtmp[:])
            else:
                if first_gp:
                    nc.gpsimd.tensor_mul(gp_acc[:], kt, shift)
                    first_gp = False
                else:
                    gtmp = accp.tile([128, 1024], BF16, tag="gtmp", bufs=2)
                    nc.gpsimd.tensor_mul(gtmp[:], kt, shift)
                    nc.gpsimd.tensor_add(gp_acc[:], gp_acc[:], gtmp[:])
        nc.vector.tensor_add(acc[:], acc[:], gp_acc[:])
        nc.scalar.activation(f_b3[:, 1024 * c:1024 * (c + 1)], acc[:], AF.Copy)

    kerns = {}
    kerns[0] = g2_chunk(0)
    kerns[1] = g2_chunk(1)
    dyn_chunk(0, kerns[0])

    # ---- S1 (emitted here so PE stays busy while DVE chews dyn chunks) ----
    s1 = pool.tile([128, HP, WP], BF16, tag="A", name="s1")
    nc.vector.memset(s1[0:64, 0, :], 0)
    nc.vector.memset(s1[64:128, HP - 1, :], 0)
    nc.vector.memset(s1[:, :, 0:1], 0)
    nc.vector.memset(s1[:, :, WP - 1:WP], 0)
    for k in range(NCHUNK):
        ps = ps_conv()
        _conv_taps(nc, ps, wt["s1T"], fev, k)
        nc.scalar.activation(s1[:, 1 + 4 * k:5 + 4 * k, 1:129], ps[:], AF.Relu)
    nc.sync.dma_start(s1[0:64, HP - 1, :], s1[64:128, 1, :])
    nc.sync.dma_start(s1[64:128, 0, :], s1[0:64, HP - 2, :])

    kerns[2] = g2_chunk(2)
    dyn_chunk(1, kerns[1])

    # ---- S2: sa = conv(s1) ----
    sa = pool.tile([128, HNP], BF16, tag="D", name="sa")
    for k in range(NCHUNK):
        ps = ps_conv()
        _conv_taps(nc, ps, wt["s2T"], s1, k)
        nc.scalar.activation(sa[:, 512 * k:512 * (k + 1)], ps[:], AF.Identity)

    kerns[3] = g2_chunk(3)
    dyn_chunk(2, kerns[2])

    # ---- channel max / sum of sa via PE transposes + DVE reduces ----
    # (transpose-mode matmuls only work from partitions 0-63 on HW, so DMA
    #  the half1 channels down first)
    sa_lo = pool.tile([64, HNP], BF16, tag="A", name="sa_lo")
    nc.sync.dma_start(sa_lo[:], sa[64:128, :])
    mx = pool.tile([128, 128], BF16)
    sm = pool.tile([128, 128], BF16)
    for t in range(16):
        tp = pp.tile([128, 512], BF16, tag="tp", bufs=2, name="tp")
        for j in range(8):
            gchunk = 8 * t + j
            src = sa if gchunk < 64 else sa_lo
            jj = gchunk % 64
            nc.tensor.matmul(tp[:, 64 * j:64 * (j + 1)],
                             src[0:64, 128 * jj:128 * (jj + 1)],
                             ident[0:64, :], is_transpose=True,
                             skip_group_check=True)
        tp3 = tp.rearrange("p (c x) -> p c x", c=8)
        nc.vector.tensor_reduce(mx[:, 8 * t:8 * (t + 1)], tp3, axis=AX.X,
                                op=OP.max)
        with nc.allow_low_precision(reason="gate-path channel mean in bf16"):
            nc.vector.tensor_reduce(sm[:, 8 * t:8 * (t + 1)], tp3, axis=AX.X,
                                    op=OP.add)

    kerns[4] = g2_chunk(4)
    dyn_chunk(3, kerns[3])

    # ---- 5x5 gate conv in row-major space via banded matmuls ----
    # comp planes land row-major ([row, w]) after dma_transpose; the 5x5 conv
    # becomes 50 accumulating K=128 matmuls with banded (shift) matrices
    # prescaled by Ew[c,i,j] on the lhsT side; column shifts are free-dim
    # offsets into zero-padded planes.
    mxT = pool.tile([128, 128], BF16)
    smT = pool.tile([128, 128], BF16)
    nc.sync.dma_start_transpose(mxT[:], mx[:])
    nc.sync.dma_start_transpose(smT[:], sm[:])
    mxTp = pool.tile([128, 132], BF16)
    smTp = pool.tile([128, 132], BF16)
    for pl, src in ((mxTp, mxT), (smTp, smT)):
        nc.vector.memset(pl[:, 0:2], 0)
        nc.vector.memset(pl[:, 130:132], 0)
        nc.vector.tensor_copy(pl[:, 2:130], src[:])

    kerns[5] = g2_chunk(5)
    dyn_chunk(4, kerns[4])
    kerns[6] = g2_chunk(6)
    dyn_chunk(5, kerns[5])

    # ---- banded Ew matmuls -> logits [row, w]; sigmoid; broadcast to split ----
    ps_log = pp.tile([128, 128], F32, tag="conv", bufs=2, name="ps_log")
    idx = 0
    for i in range(5):
        for j in range(5):
            for cc in range(2):
                rhs = (mxTp if cc == 0 else smTp)[:, j:j + 128]
                nc.tensor.matmul(ps_log[:], bands[:, idx, :], rhs,
                                 start=(idx == 0), stop=(idx == 49),
                                 skip_group_check=True)
                idx += 1
    se_rows = pool.tile([128, 128], BF16)
    nc.scalar.activation(se_rows[:], ps_log[:], AF.Sigmoid, bias=eb_f)
    # flatten each half of se_rows to one partition, then replicate
    se = pool.tile([128, HNP], BF16, tag="A", name="se")
    flat0 = pool.tile([1, HNP], BF16, tag="flat", name="flat0")
    nc.sync.dma_start(flat0[0:1, :].rearrange("p (r w) -> p r w", r=64),
                      se_rows[0:64, :])
    nc.sync.dma_start(se[0:64, :], flat0[:].partition_broadcast(64))
    flat1 = pool.tile([1, HNP], BF16, tag="flat", name="flat1")
    nc.sync.dma_start(flat1[0:1, :].rearrange("p (r w) -> p r w", r=64),
                      se_rows[64:128, :])
    nc.sync.dma_start(se[64:128, :], flat1[:].partition_broadcast(64))

    kerns[7] = g2_chunk(7)
    dyn_chunk(6, kerns[6])

    f_e = pool.tile([128, HNP], BF16, tag="D", name="f_e")
    nc.vector.scalar_tensor_tensor(f_e[:], se[:], 1.0, fev[:, 1:65, 1:129],
                                   op0=OP.add, op1=OP.mult)

    dyn_chunk(7, kerns[7])

    # ---- Fw 1x1 fusion conv -> output ----
    outp = ctx.enter_context(tc.tile_pool(name="out", bufs=1))
    for half in range(2):
        b = half * 64
        for k in range(NCHUNK):
            ps = ps_conv()
            nc.tensor.matmul(ps[:], fwAT[b:b + 64, :],
                             f_e[b:b + 64, 512 * k:512 * (k + 1)],
                             start=True, stop=False, tile_position=(b, 0),
                             skip_group_check=True)
            nc.tensor.matmul(ps[:], fwBT[b:b + 64, :],
                             f_b3[b:b + 64, 512 * k:512 * (k + 1)],
                             start=False, stop=True, tile_position=(b, 0),
                             skip_group_check=True)
            ot = outp.tile([128, 512], F32, tag="ot")
            nc.scalar.activation(ot[:], ps[:], AF.Copy)
            nc.sync.dma_start(io["res"][:, half * HNP + 512 * k:
                                        half * HNP + 512 * (k + 1)], ot[:])


def _prep_weights(inp):
    """Host-side weight transforms -> DRAM tensors for the kernel."""
    f32, o = np.float32, {}

    def dup(a):  # duplicate 64-partition data onto both partition halves
        return np.concatenate([a, a], axis=0)

    def conv_taps_T(Wc):  # [out, in, 3, 3] -> [128, 9, 64] (lhsT per tap, dup)
        t = np.transpose(Wc.reshape(64, 64, 9), (1, 2, 0))  # [cin, tap, cout]
        return dup(np.ascontiguousarray(t)).astype(bf)

    for name, key in (("wt1T", "Wt1"), ("wt2T", "Wt2"), ("wt3T", "Wt3"),
                      ("s1T", "S1"), ("s2T", "S2"), ("g1T", "G1")):
        o[name] = conv_taps_T(inp[key].astype(f32))

    g2 = np.asarray(inp["G2"], f32).reshape(64, 9, 64)   # [cout, tap, cin]
    g2T = np.transpose(g2, (2, 1, 0))                     # [cin, tap, cout]
    o["g2T"] = dup(g2T).astype(bf)

    rw = np.asarray(inp["Rw"], f32).reshape(64)           # [cin]
    o["rwT"] = dup(np.repeat(rw[:, None], 64, 1)).astype(bf)

    fw = np.asarray(inp["Fw"], f32).reshape(128, 128)     # [out, in]
    o["fwAT"] = dup(fw[:, 0:64].T).astype(bf)
    o["fwBT"] = dup(fw[:, 64:128].T).astype(bf)

    ew = np.asarray(inp["Ew"], f32).reshape(2, 5, 5)
    bands = np.zeros((128, 50, 128), f32)
    idx = 0
    for i in range(5):
        for j in range(5):
            for cc in range(2):
                v = ew[cc, i, j] * (1.0 / 64.0 if cc == 1 else 1.0)
                bands[:, idx, :] = v * np.eye(128, k=2 - i)
                idx += 1
    o["bands"] = bands.astype(bf)

    o["a1T"] = (np.asarray(inp["A1"], f32).T / 16384.0).astype(bf)
    o["a2T"] = np.asarray(inp["A2"], f32).T.astype(bf)
    o["b1p"] = np.asarray(inp["b1"], f32)[:, None].copy()
    o["b2p"] = np.asarray(inp["b2"], f32)[:, None].copy()
    o["g1bp"] = dup(np.asarray(inp["g1b"], f32)[:, None]).copy()
    g2b = np.asarray(inp["g2b"], f32).reshape(64, 9).copy()
    g2b[:, 4] += 1.0                                      # fold "+ f_b" residual
    o["g2bp"] = dup(g2b)
    o["ident"] = dup(np.eye(64, dtype=f32)).astype(bf)
    return o


_CACHE = {}


def _get_program(rb_f, eb_f):
    key = (rb_f, eb_f)
    if key in _CACHE:
        return _CACHE[key]
    nc = bacc.Bacc("TRN2", target_bir_lowering=False, debug=False,
                   num_devices=1, enable_asserts=True)
    io = {}
    io["fe"] = nc.dram_tensor("fe", [64, NP], F32, kind="ExternalInput").ap()
    io["fb"] = nc.dram_tensor("fb", [64, NP], F32, kind="ExternalInput").ap()
    for name in ("wt1T", "wt2T", "wt3T", "s1T", "s2T", "g1T", "g2T"):
        io[name] = nc.dram_tensor(name, [128, 9, 64], BF16, kind="ExternalInput").ap()
    for name, shape, dt in (("rwT", [128, 64], BF16), ("fwAT", [128, 128], BF16),
                            ("fwBT", [128, 128], BF16), ("bands", [128, 50, 128], BF16),
                            ("a1T", [64, 128], BF16), ("a2T", [128, 64], BF16),
                            ("ident", [128, 64], BF16), ("b1p", [128, 1], F32),
                            ("b2p", [64, 1], F32), ("g1bp", [128, 1], F32),
                            ("g2bp", [128, 9], F32)):
        io[name] = nc.dram_tensor(name, shape, dt, kind="ExternalInput").ap()
    io["res"] = nc.dram_tensor("res", [128, NP], F32, kind="ExternalOutput").ap()

    with ExitStack() as ctx:
        tc = ctx.enter_context(tile.TileContext(nc))
        build_kernel(ctx, tc, io, rb_f, eb_f)
    nc.finalize()
    _CACHE[key] = nc
    return nc


def _run(inputs, trace=False, **kw):
    rb_f = float(np.asarray(inputs["rb"]).reshape(-1)[0])
    eb_f = float(np.asarray(inputs["eb"]).reshape(-1)[0])
    nc = _get_program(rb_f, eb_f)
    wts = _prep_weights(inputs)
    fe = np.asarray(inputs["f_event"], np.float32).reshape(8, 64, NP)
    fbl = np.asarray(inputs["f_blur"], np.float32).reshape(8, 64, NP)
    in_maps = []
    for core in range(8):
        m = {"fe": np.ascontiguousarray(fe[core]),
             "fb": np.ascontiguousarray(fbl[core])}
        m.update(wts)
        in_maps.append(m)
    res = run_bass_kernel_spmd(nc, in_maps, core_ids=list(range(8)),
                               trace=trace, **kw)
    out = np.stack([np.asarray(r["res"]).reshape(128, H, W)
                    for r in res.results], axis=0)
    return out.astype(np.float32), res


def _make_timed_runner(inputs):
    """Build a reusable jitted 8-core executable for timing (no donation)."""
    import jax
    from jax.sharding import Mesh, PartitionSpec
    from jax.experimental.shard_map import shard_map
    from concourse import bass2jax
    rb_f = float(np.asarray(inputs["rb"]).reshape(-1)[0])
    eb_f = float(np.asarray(inputs["eb"]).reshape(-1)[0])
    nc = _get_program(rb_f, eb_f)
    bass2jax.install_neuronx_cc_hook()
    wts = _prep_weights(inputs)
    fe = np.asarray(inputs["f_event"], np.float32).reshape(8, 64, NP)
    fbl = np.asarray(inputs["f_blur"], np.float32).reshape(8, 64, NP)
    in_maps = []
    for core in range(8):
        m = {"fe": np.ascontiguousarray(fe[core]),
             "fb": np.ascontiguousarray(fbl[core])}
        m.update(wts)
        in_maps.append(m)

    import concourse.mybir as mybir_m
    partition_name = (nc.partition_id_tensor.name if nc.partition_id_tensor
                      else None)
    in_names, out_names, out_avals = [], [], []
    for alloc in nc.m.functions[0].allocations:
        if not isinstance(alloc, mybir_m.MemoryLocationSet):
            continue
        name = alloc.memorylocations[0].name
        if alloc.kind == "ExternalInput":
            if name != partition_name:
                in_names.append(name)
        elif alloc.kind == "ExternalOutput":
            out_names.append(name)
            out_avals.append(jax.core.ShapedArray(tuple(alloc.tensor_shape),
                                                  mybir_m.dt.np(alloc.dtype)))

    def _body(*args):
        operands = list(args)
        if partition_name is not None:
            operands.append(bass2jax.partition_id_tensor())
        outs = bass2jax._bass_exec_p.bind(
            *operands, out_avals=tuple(out_avals),
            in_names=tuple(in_names + out_names +
                           ([partition_name] if partition_name else [])),
            out_names=tuple(out_names),
            lowering_input_output_aliases=(),
            sim_require_finite=True, sim_require_nnan=True, nc=nc)
        return tuple(outs)

    devices = jax.devices()[:8]
    mesh = Mesh(np.array(devices), ("core",))
    n_in = len(in_names) + len(out_names)
    sharded = jax.jit(shard_map(_body, mesh=mesh,
                                in_specs=(PartitionSpec("core"),) * n_in,
                                out_specs=(PartitionSpec("core"),) * len(out_names),
                                check_rep=False), keep_unused=True)
    concat_in = [np.concatenate([np.asarray(in_maps[c][n]) for c in range(8)], 0)
                 for n in in_names]
    concat_in += [np.zeros((8 * a.shape[0],) + a.shape[1:], a.dtype)
                  for a in out_avals]
    def run():
        return jax.block_until_ready(sharded(*concat_in))
    return run, out_names


def kernel(**inputs):
    return _run(inputs)[0]
